# revision 6
# baseline (speedup 1.0000x reference)
"""BERT self-attention on 8 TRN2 NeuronCores, data-parallel over batch.

Full inputs in, full outputs out. Each core processes one batch element:
  qkv = x @ Wqkv + b ; per-head softmax((q k^T)/sqrt(hd) + mask) @ v ; @ Wp + b

Layout strategy (per core, S=1024, D=1024, 16 heads x 64):
  - x is PE-transposed to xT [D, S] once.
  - q,k are produced TRANSPOSED (qkT [2048, S]) so per-head scores come out
    as scoresT [Sk, Sq] (keys on partitions): lhsT = kT_h (zero-padded to
    K=128), rhs = qT pair tile.
  - softmax: exp on ScalarE (scale=1/8 fused); no max-subtraction needed
    (|scores/8| <~ 6). The denominator comes free from an appended
    ones-column in the PV matmul rhs ([v | 1]); the attention mask is
    applied by zeroing masked key ROWS of [v | 1].
  - PV: out[Sq,65] accumulated over Sk chunks; divide by the ones-column.
  - attn output PE-transposed, then proj matmul with Wp.
"""

import numpy as np

P = 128
S = 1024
D = 1024
N_H = 16
HD = 64  # head dim
N_CORES = 8

# "bfloat16" | "float32" | "float32r"
COMPUTE_DT = "bfloat16"


def build_bass(compute_dt_name=None):
    import concourse.mybir as mybir
    import concourse.tile as tile
    from concourse import bacc
    from concourse.masks import make_identity
    from contextlib import ExitStack

    cdt = getattr(mybir.dt, compute_dt_name or COMPUTE_DT)
    f32 = mybir.dt.float32
    i32 = mybir.dt.int32
    AF = mybir.ActivationFunctionType
    ALU = mybir.AluOpType

    nc = bacc.Bacc(None, target_bir_lowering=False)

    x_d = nc.declare_dram_parameter("x", [S, D], f32, isOutput=False)
    mask_d = nc.declare_dram_parameter("attention_mask", [S], i32, isOutput=False)
    wqkv_d = nc.declare_dram_parameter("Wqkv", [D, 3 * D], f32, isOutput=False)
    bqkv_d = nc.declare_dram_parameter("bqkv", [3 * D], f32, isOutput=False)
    wp_d = nc.declare_dram_parameter("Wp", [D, D], f32, isOutput=False)
    bp_d = nc.declare_dram_parameter("bp", [D], f32, isOutput=False)
    out_d = nc.declare_dram_parameter("out", [S, D], f32, isOutput=True)

    x_v = x_d.rearrange("(po pi) d -> pi po d", pi=P)      # [128, 8, 1024]
    mask_v = mask_d.rearrange("(po pi) -> pi po", pi=P)    # [128, 8]
    bqk_v = bqkv_d[: 2 * D].rearrange("(po pi) -> pi po", pi=P)  # [128, 16]
    out_v = out_d.rearrange("(po pi) d -> pi po d", pi=P)

    SPO = S // P   # 8 seq chunks
    DPO = D // P   # 8 feature chunks
    NQK = 2 * D // P  # 16 qk feature chunks

    with ExitStack() as top:
        tc = top.enter_context(tile.TileContext(nc))
        const = top.enter_context(tc.tile_pool(name="const", bufs=1))
        psum = top.enter_context(tc.tile_pool(name="psum", bufs=6, space="PSUM"))

        # --- constants ---
        ident = const.tile([P, P], cdt)
        make_identity(nc, ident)

        bqk_sb = const.tile([P, NQK], f32)
        nc.sync.dma_start(bqk_sb[:], bqk_v)

        mask_i = const.tile([P, SPO], i32)
        nc.sync.dma_start(mask_i[:], mask_v)
        mask_f = const.tile([P, SPO], f32)
        # 1.0 where mask!=0 else 0.0
        nc.vector.tensor_scalar(mask_f[:], mask_i[:], 0, None, ALU.not_equal)

        # bias rows replicated across all 128 partitions via doubling DMA
        bv_bc = const.tile([P, D], f32)   # viewed as [P, 16, 64] at use site
        nc.sync.dma_start(bv_bc[:1, :], bqkv_d[None, 2 * D:])
        bp_bc = const.tile([P, D], f32)
        nc.sync.dma_start(bp_bc[:1, :], bp_d[None, :])
        step = 1
        while step < P:
            nc.sync.dma_start(bv_bc[step: 2 * step, :], bv_bc[:step, :])
            nc.sync.dma_start(bp_bc[step: 2 * step, :], bp_bc[:step, :])
            step *= 2

        def psum_tile():
            return psum.tile([P, 512], f32, tag="ps", name="ps")

        def psum_tr_tile():
            return psum.tile([P, P], cdt, tag="ps", name="pst")

        # --- phase 1: load x, cast, transpose -> xT [128, 8(dpo), 1024(s)] ---
        p1 = ExitStack()
        xT_pool = top.enter_context(tc.tile_pool(name="xT", bufs=1))
        xT = xT_pool.tile([P, DPO, S], cdt)
        with p1:
            xstage = p1.enter_context(tc.tile_pool(name="xstage", bufs=2))
            xbf_pool = p1.enter_context(tc.tile_pool(name="xbf", bufs=2))
            for po in range(SPO):
                xs = xstage.tile([P, D], f32, tag="xs")
                nc.sync.dma_start(xs[:], x_v[:, po, :])
                xb = xbf_pool.tile([P, D], cdt, tag="xb")
                nc.gpsimd.tensor_copy(xb[:], xs[:])
                for dblk in range(DPO):
                    pt = psum_tr_tile()
                    nc.tensor.transpose(
                        pt[:], xb[:, dblk * P: (dblk + 1) * P], ident[:]
                    )
                    nc.scalar.copy(
                        xT[:, dblk, po * P: (po + 1) * P], pt[:]
                    )

        # --- phase 2: qkT = (Wqk)^T @ xT  -> [128, 16(m), 1024(s)] + bias ---
        qkT_pool = top.enter_context(tc.tile_pool(name="qkT", bufs=1))
        qkT = qkT_pool.tile([P, NQK, S], cdt)
        with ExitStack() as p2:
            wstage = p2.enter_context(tc.tile_pool(name="wqk_stage", bufs=2))
            wqk_pool = p2.enter_context(tc.tile_pool(name="wqk", bufs=1))
            wqk = wqk_pool.tile([P, DPO, 2 * D], cdt)
            for k in range(DPO):
                ws = wstage.tile([P, 2 * D], f32, tag="ws")
                nc.sync.dma_start(ws[:], wqkv_d[k * P: (k + 1) * P, : 2 * D])
                nc.gpsimd.tensor_copy(wqk[:, k, :], ws[:])
            for m in range(NQK):
                for half in range(2):
                    pt = psum_tile()
                    for k in range(DPO):
                        nc.tensor.matmul(
                            pt[:],
                            wqk[:, k, m * P: (m + 1) * P],
                            xT[:, k, half * 512: (half + 1) * 512],
                            start=(k == 0),
                            stop=(k == DPO - 1),
                        )
                    nc.vector.tensor_scalar_add(
                        qkT[:, m, half * 512: (half + 1) * 512],
                        pt[:],
                        bqk_sb[:, m: m + 1],
                    )

        # --- phase 3: v_ext [128, 8(s po), 16(h), 65] = (x @ Wv + bv | 1) * maskbit
        vext_pool = top.enter_context(tc.tile_pool(name="vext", bufs=1))
        v_ext = vext_pool.tile([P, SPO, N_H, HD + 1], cdt)
        with ExitStack() as p3:
            wvstage = p3.enter_context(tc.tile_pool(name="wv_stage", bufs=2))
            wv_pool = p3.enter_context(tc.tile_pool(name="wv", bufs=1))
            wv = wv_pool.tile([P, DPO, D], cdt)
            for k in range(DPO):
                ws = wvstage.tile([P, D], f32, tag="wvs")
                nc.sync.dma_start(ws[:], wqkv_d[k * P: (k + 1) * P, 2 * D:])
                nc.gpsimd.tensor_copy(wv[:, k, :], ws[:])
            bv_v = bv_bc[:].rearrange("p (h e) -> p h e", e=HD)  # [P, 16, 64]
            for m in range(SPO):
                for half in range(2):
                    pt = psum_tile()
                    for k in range(DPO):
                        nc.tensor.matmul(
                            pt[:],
                            xT[:, k, m * P: (m + 1) * P],
                            wv[:, k, half * 512: (half + 1) * 512],
                            start=(k == 0),
                            stop=(k == DPO - 1),
                        )
                    h0 = half * (N_H // 2)
                    h1 = h0 + N_H // 2
                    nc.vector.tensor_tensor(
                        v_ext[:, m, h0:h1, :HD],
                        pt[:].rearrange("p (h e) -> p h e", e=HD),
                        bv_v[:, h0:h1, :],
                        ALU.add,
                    )
                # ones column, then zero out masked key rows (whole [16,65] row)
                nc.vector.memset(v_ext[:, m, :, HD: HD + 1], 1.0)
                nc.vector.tensor_scalar_mul(
                    v_ext[:, m, :, :], v_ext[:, m, :, :], mask_f[:, m: m + 1]
                )

        # --- phase 4: attention per head ---
        attn_pool = top.enter_context(tc.tile_pool(name="attn", bufs=1))
        attn_out = attn_pool.tile([P, SPO, D], cdt)
        with ExitStack() as p4:
            kpad_pool = p4.enter_context(tc.tile_pool(name="kpad", bufs=2))
            expT_pool = p4.enter_context(tc.tile_pool(name="expT", bufs=2))
            rcp_pool = p4.enter_context(tc.tile_pool(name="rcp", bufs=4))
            for h in range(N_H):
                off = HD * (h % 2)
                doff = HD - off  # offset of the dead half
                kp = kpad_pool.tile([P, S], cdt, tag="kp")
                nc.vector.memset(kp[doff: doff + HD, :], 0.0)
                nc.sync.dma_start(
                    kp[off: off + HD, :], qkT[off: off + HD, 8 + h // 2, :]
                )
                eT = expT_pool.tile([P, SPO, S], cdt, tag="eT")
                for sk in range(SPO):
                    for half in range(2):
                        pt = psum_tile()
                        nc.tensor.matmul(
                            pt[:],
                            kp[:, sk * P: (sk + 1) * P],
                            qkT[:, h // 2, half * 512: (half + 1) * 512],
                            start=True,
                            stop=True,
                        )
                        nc.scalar.activation(
                            eT[:, sk, half * 512: (half + 1) * 512],
                            pt[:],
                            AF.Exp,
                            scale=1.0 / np.sqrt(HD),
                        )
                for sq in range(SPO):
                    pt = psum_tile()
                    po = pt[:, : HD + 1]
                    for sk in range(SPO):
                        nc.tensor.matmul(
                            po,
                            eT[:, sk, sq * P: (sq + 1) * P],
                            v_ext[:, sk, h, :],
                            start=(sk == 0),
                            stop=(sk == SPO - 1),
                        )
                    rcp = rcp_pool.tile([P, 1], f32, tag="rcp")
                    nc.vector.reciprocal(rcp[:], po[:, HD: HD + 1])
                    nc.vector.tensor_scalar_mul(
                        attn_out[:, sq, h * HD: (h + 1) * HD],
                        po[:, :HD],
                        rcp[:],
                    )

        # --- phase 5: transpose attn_out -> attnT [128, 8(dpo), 1024(s)] ---
        attnT_pool = top.enter_context(tc.tile_pool(name="attnT", bufs=1))
        attnT = attnT_pool.tile([P, DPO, S], cdt)
        for po in range(SPO):
            for dblk in range(DPO):
                pt = psum_tr_tile()
                nc.tensor.transpose(
                    pt[:], attn_out[:, po, dblk * P: (dblk + 1) * P], ident[:]
                )
                nc.scalar.copy(attnT[:, dblk, po * P: (po + 1) * P], pt[:])

        # --- phase 6: out = attn @ Wp + bp ---
        with ExitStack() as p6:
            wpstage = p6.enter_context(tc.tile_pool(name="wp_stage", bufs=2))
            wp_pool = p6.enter_context(tc.tile_pool(name="wp", bufs=1))
            ystage = p6.enter_context(tc.tile_pool(name="y", bufs=2))
            wp = wp_pool.tile([P, DPO, D], cdt)
            for k in range(DPO):
                ws = wpstage.tile([P, D], f32, tag="wps")
                nc.sync.dma_start(ws[:], wp_d[k * P: (k + 1) * P, :])
                nc.gpsimd.tensor_copy(wp[:, k, :], ws[:])
            for m in range(SPO):
                y = ystage.tile([P, D], f32, tag="y")
                for half in range(2):
                    pt = psum_tile()
                    for k in range(DPO):
                        nc.tensor.matmul(
                            pt[:],
                            attnT[:, k, m * P: (m + 1) * P],
                            wp[:, k, half * 512: (half + 1) * 512],
                            start=(k == 0),
                            stop=(k == DPO - 1),
                        )
                    nc.vector.tensor_add(
                        y[:, half * 512: (half + 1) * 512],
                        pt[:],
                        bp_bc[:, half * 512: (half + 1) * 512],
                    )
                nc.sync.dma_start(out_v[:, m, :], y[:])

    return nc


_CACHE = {}


def _get_compiled(dt_name=None):
    key = dt_name or COMPUTE_DT
    if key not in _CACHE:
        nc = build_bass(key)
        nc.compile()
        _CACHE[key] = nc
    return _CACHE[key]


def kernel(x, attention_mask, Wqkv, bqkv, Wp, bp):
    from concourse.bass_utils import run_bass_kernel_spmd

    x = np.asarray(x, dtype=np.float32)
    attention_mask = np.asarray(attention_mask, dtype=np.int32)
    Wqkv = np.asarray(Wqkv, dtype=np.float32)
    bqkv = np.asarray(bqkv, dtype=np.float32)
    Wp = np.asarray(Wp, dtype=np.float32)
    bp = np.asarray(bp, dtype=np.float32)

    nc = _get_compiled()
    in_maps = [
        {
            "x": x[b],
            "attention_mask": attention_mask[b],
            "Wqkv": Wqkv,
            "bqkv": bqkv,
            "Wp": Wp,
            "bp": bp,
        }
        for b in range(N_CORES)
    ]
    res = run_bass_kernel_spmd(nc, in_maps, core_ids=list(range(N_CORES)))
    return np.stack([res.results[b]["out"] for b in range(N_CORES)])
